# revision 20
# baseline (speedup 1.0000x reference)
"""DGCNN (nn_DGCNN_50594714747409) Bass/TRN2 kernel — 8-core data parallel.

Contract: kernel(**inputs) takes the FULL unsharded inputs (as produced by
setup_inputs()) and returns the FULL [16, 101] output. Internally shards the
batch (16) across 8 NeuronCores (2 samples/core), runs one SPMD Bass program
per core via bass_utils.run_bass_kernel_spmd, and concatenates the outputs.

Algorithm notes (refactor of the reference; error budget rel<2e-2):
  * EdgeConv: max_k(lrelu(bn(W @ [x_j - x_i; x_i]))) with bn scale > 0 and
    lrelu monotone ==> lrelu(bn(max_k(Wn@x_j) + (Wc-Wn)@x_i)). BN folded into
    conv weights on the host.
  * kNN: top-20 of s[i,j] = x_i.x_j - 0.5||x_j||^2 (same per-row order as the
    reference's -||x_i - x_j||^2). Distance matmuls run in f32r on the PE.
    Top-20 selection via int32 packing: the PSUM eviction (Act engine)
    quantizes M = round(d*S) to i32 (S computed on-device per sample/layer
    from max||x_j||^2), gpsimd packs (M<<10)+j, and the DVE needs only 5
    passes (3x max8 + 2x match_replace) over the packed values bitcast to
    f32 (nonneg IEEE floats order like their bit patterns). Indices come
    from the low 10 bits of the selected values - no max_index scans.
  * Neighbor gather runs in fp16 (a = Wn@x stored fp16 in HBM, SWDGE
    dma_gather, 20-way strided max-reduce on DVE). fp16 is only on this
    value path; distances/features stay f32.
  * Bias + LeakyReLU fused into the output transpose: PE accumulates
    (Wb@x)^T + z^T in PSUM, Act engine evicts with Lrelu(alpha=0.2) and the
    per-channel bias br as the per-partition activation bias.
  * conv5 in f32r + fused global max/mean pool (mean via activation
    accum_out); FC head batched over both samples per core, f32r matmuls.
  * The two samples per core are interleaved layer-by-layer so one sample's
    gather/DMA phase overlaps the other's top-K DVE phase.
"""

import numpy as np

import concourse.bass as bass
import concourse.bacc as bacc
import concourse.mybir as mybir
from concourse.tile import TileContext
from concourse import bass_utils

F32 = mybir.dt.float32
F32R = mybir.dt.float32r
F16 = mybir.dt.float16
U32 = mybir.dt.uint32
I32 = mybir.dt.int32
I16 = mybir.dt.int16
ALU = mybir.AluOpType
ACTF = mybir.ActivationFunctionType

N = 1024
KNN = 20
EPS = 1e-5
LAYERS = [(3, 64), (64, 64), (64, 128), (128, 256)]  # (C_in, O)
NCORES = 8
BPC = 2  # samples per core
MAGIC = float(2 ** 23)  # f32 magic: y = d*S + 2^23 + 2^19 rounds to int M


def build_nc(bpc=BPC):
    nc = bacc.Bacc("TRN2", target_bir_lowering=False, debug=False)

    # ---- I/O ----
    x_in = nc.dram_tensor("x", [bpc, 3, N], F32R, kind="ExternalInput")
    win = {}
    for l, (C, O) in enumerate(LAYERS, 1):
        win[f"wnt{l}"] = nc.dram_tensor(f"wnt{l}", [C, O], F16, kind="ExternalInput")
        win[f"wbt{l}"] = nc.dram_tensor(f"wbt{l}", [C, O], F16, kind="ExternalInput")
        win[f"brt{l}"] = nc.dram_tensor(f"brt{l}", [O, 1], F32, kind="ExternalInput")
    win["w5t"] = nc.dram_tensor("w5t", [4, 128, N], F32R, kind="ExternalInput")
    win["b5c"] = nc.dram_tensor("b5c", [128, 8], F32, kind="ExternalInput")
    win["w6t"] = nc.dram_tensor("w6t", [16, 128, 512], F32R, kind="ExternalInput")
    win["b6r"] = nc.dram_tensor("b6r", [1, 512], F32R, kind="ExternalInput")
    win["w7t"] = nc.dram_tensor("w7t", [4, 128, 256], F32R, kind="ExternalInput")
    win["b7r"] = nc.dram_tensor("b7r", [1, 256], F32R, kind="ExternalInput")
    win["w8t"] = nc.dram_tensor("w8t", [2, 128, 101], F32, kind="ExternalInput")
    win["b8r"] = nc.dram_tensor("b8r", [1, 101], F32, kind="ExternalInput")
    ident_in = nc.dram_tensor("ident", [128, 128], F32, kind="ExternalInput")
    ones_in = nc.dram_tensor("ones", [1, 128], F32R, kind="ExternalInput")
    mhalf_in = nc.dram_tensor("mhalf", [128, 1], F32R, kind="ExternalInput")
    iota_in = nc.dram_tensor("iota", [128, N], I32, kind="ExternalInput")
    qb_in = nc.dram_tensor("qb", [128, 1], F32, kind="ExternalInput")
    out = nc.dram_tensor("out", [bpc, 101], F32, kind="ExternalOutput")

    with TileContext(nc) as tc:
        import contextlib
        ctx = contextlib.ExitStack()
        with ctx:
            wpool = ctx.enter_context(tc.tile_pool(name="w", bufs=1))
            pool = ctx.enter_context(tc.tile_pool(name="sb", bufs=2))
            big = ctx.enter_context(tc.tile_pool(name="big", bufs=1))
            psum = ctx.enter_context(tc.tile_pool(name="ps", bufs=2, space="PSUM"))
            dram = ctx.enter_context(tc.tile_pool(name="dr", bufs=2, space="DRAM"))

            # ---- stage weights/constants into SBUF ----
            ident = wpool.tile([128, 128], F32, tag="ident")
            nc.sync.dma_start(ident[:], ident_in[:])
            ones = wpool.tile([1, 128], F32R, tag="ones")
            nc.sync.dma_start(ones[:], ones_in[:])
            mhalf = wpool.tile([128, 1], F32R, tag="mhalf")
            nc.sync.dma_start(mhalf[:], mhalf_in[:])
            iota = wpool.tile([128, N], I32, tag="iota")
            nc.sync.dma_start(iota[:], iota_in[:])
            qb = wpool.tile([128, 1], F32, tag="qb")
            nc.sync.dma_start(qb[:], qb_in[:])
            wsb = {}
            for l, (C, O) in enumerate(LAYERS, 1):
                wsb[f"wnt{l}"] = wpool.tile([C, O], F16, tag=f"wnt{l}", name=f"wnt{l}")
                wsb[f"wbt{l}"] = wpool.tile([C, O], F16, tag=f"wbt{l}", name=f"wbt{l}")
                hb = (O + 127) // 128
                wsb[f"brt{l}"] = wpool.tile([min(O, 128), hb], F32,
                                            tag=f"brt{l}", name=f"brt{l}")
                nc.sync.dma_start(wsb[f"wnt{l}"][:], win[f"wnt{l}"][:])
                nc.sync.dma_start(wsb[f"wbt{l}"][:], win[f"wbt{l}"][:])
                nc.sync.dma_start(
                    wsb[f"brt{l}"][:],
                    win[f"brt{l}"][:].rearrange("(h p) o -> p (h o)", h=hb))
            for k, shp, kdt in [("w5t", [128, 4, N], F32R), ("b5c", [128, 8], F32),
                                ("b6r", [1, 512], F32R),
                                ("w7t", [128, 4, 256], F32R),
                                ("b7r", [1, 256], F32R),
                                ("w8t", [128, 2, 101], F32),
                                ("b8r", [1, 101], F32)]:
                wsb[k] = wpool.tile(shp, kdt, tag=k, name=k)
                if len(shp) == 3:
                    nc.sync.dma_start(wsb[k][:], win[k][:].rearrange("a b c -> b a c"))
                else:
                    nc.sync.dma_start(wsb[k][:], win[k][:])

            # per-sample persistent feature tiles (xc = concat of layer outputs)
            # xc[s][0]: ch 0-127 (x1 | x2), xc[s][1]: x3, xc[s][2:4]: x4
            xc = [[big.tile([128, N], F32R, tag=f"xc{s}_{t}", name=f"xc{s}_{t}")
                   for t in range(4)] for s in range(bpc)]
            x2own = [big.tile([64, N], F32R, tag=f"x2own{s}", name=f"x2own{s}")
                     for s in range(bpc)]
            p2 = big.tile([128, 16, bpc], F32R, tag="p2")  # pooled [max|mean]

            def edge_layer(s, l, cur, C, O, dests):
                """cur: AP [C, N] f32 input features. dests[h]: list of
                (tile, row_off) for 128-row chunk h of the [O, N] output."""
                t = f"{s}"
                wnt, wbt = wsb[f"wnt{l}"], wsb[f"wbt{l}"]
                brt = wsb[f"brt{l}"]
                # fp16 copy of cur for the a/b value-path matmuls
                cur16 = pool.tile([C, N], F16, tag="cur16" + t, bufs=1)
                nc.scalar.copy(cur16[:], cur)
                sq = pool.tile([C, N], F32R, tag="sq" + t, bufs=1)
                nc.scalar.activation(sq[:], cur, ACTF.Square)
                negxx = pool.tile([1, N], F32R, tag="negxx" + t)
                for jc in range(2):
                    xx_ps = psum.tile([1, 512], F32, tag="aux")
                    nc.tensor.matmul(xx_ps[:], mhalf[:C, :],
                                     sq[:, jc * 512:(jc + 1) * 512],
                                     start=True, stop=True)
                    nc.scalar.copy(negxx[:, jc * 512:(jc + 1) * 512], xx_ps[:])
                # per-(sample,layer) quantization scale S = 2^18 / max||x||^2
                # (|d| <= 1.5*max||x||^2, so |d*S| < 1.5*2^18 < 2^19-margin)
                m0 = pool.tile([1, 4], F32, tag="m0" + t)
                nc.vector.tensor_reduce(m0[:, 0:1], negxx[:].bitcast(F32),
                                        axis=mybir.AxisListType.X, op=ALU.min)
                # m0 = -0.5*max||x||^2  (<= 0);  S = 2^18 / (0.5*max||x||^2+eps)
                nc.vector.tensor_scalar(out=m0[:, 1:2], in0=m0[:, 0:1],
                                        scalar1=-1.0, scalar2=1e-12,
                                        op0=ALU.mult, op1=ALU.add)
                nc.vector.reciprocal(m0[:, 2:3], m0[:, 1:2])
                nc.vector.tensor_scalar(out=m0[:, 3:4], in0=m0[:, 2:3],
                                        scalar1=float(2 ** 17), scalar2=None,
                                        op0=ALU.mult)
                s_ps = psum.tile([128, 1], F32, tag="aux")
                nc.tensor.matmul(s_ps[:], ones[:, :].bitcast(F32),
                                 m0[:, 3:4], start=True, stop=True)
                s_sb = pool.tile([128, 1], F32, tag="ssb" + t)
                nc.scalar.copy(s_sb[:], s_ps[:])

                idx = pool.tile([128, 8, 24], U32, tag="idx" + t)
                # SWDGE gather needs elem_size % 256 bytes: fp16 only for O>=128
                adt = F16 if O >= 128 else F32
                at_dr = dram.tile([N, O], adt, tag="at_dr" + t)
                for it in range(8):
                    isl = slice(it * 128, (it + 1) * 128)
                    d_ps = psum.tile([128, N], F32, tag="dist")
                    for jc in range(2):
                        jsl = slice(jc * 512, (jc + 1) * 512)
                        nc.tensor.matmul(d_ps[:, jsl], cur[:, isl],
                                         cur[:, jsl], start=True, stop=False)
                        nc.tensor.matmul(d_ps[:, jsl], ones[:, :128],
                                         negxx[:, jsl], start=False, stop=True)
                    # quantize to i32 on eviction: M = round(d*S) + 2^19
                    dsb = pool.tile([128, N], I32, tag="dsb" + t)
                    nc.scalar.activation(dsb[:], d_ps[:], ACTF.Identity,
                                         bias=qb[:, 0:1], scale=s_sb[:, 0:1])
                    # pack (M<<10)|j on DVE (int ALU: shift+or, manual int
                    # imm -- scalar_tensor_tensor lowers float imms only);
                    # bitcast to f32 for max8 ordering (nonneg IEEE floats
                    # order like their bit patterns)
                    eng = nc.vector
                    eng.add_instruction(mybir.InstTensorScalarPtr(
                        name=eng.bass.get_next_instruction_name(),
                        is_scalar_tensor_tensor=True,
                        op0=ALU.arith_shift_left, op1=ALU.bitwise_or,
                        ins=[eng.lower_ap(dsb[:]),
                             mybir.ImmediateValue(dtype=I32, value=10),
                             eng.lower_ap(iota[:])],
                        outs=[eng.lower_ap(dsb[:])]))
                    dsf = dsb[:].bitcast(F32)
                    # exact top-24 >= top-20 per row: 3 rounds of max8
                    mx = pool.tile([128, 24], F32, tag="mx" + t)
                    for r in range(3):
                        nc.vector.max(mx[:, r * 8:(r + 1) * 8], dsf)
                        if r < 2:
                            nc.vector.match_replace(dsf, mx[:, r * 8:(r + 1) * 8],
                                                    dsf, 0.0)
                    # idx = packed & 1023 (low 10 bits)
                    nc.vector.tensor_scalar(
                        out=idx[:, it, :], in0=mx[:].bitcast(U32),
                        scalar1=1023, scalar2=None, op0=ALU.bitwise_and)
                    # aT (fp16, to HBM for the gather)
                    a_ps = psum.tile([128, O], F32, tag="aux")
                    nc.tensor.matmul(a_ps[:], cur16[:, isl], wnt[:],
                                     start=True, stop=True)
                    a_st = pool.tile([128, O], adt, tag="a_st" + t)
                    nc.scalar.copy(a_st[:], a_ps[:])
                    nc.sync.dma_start(at_dr[isl, :], a_st[:])
                # J wrap for dma_gather: jA[r, it*160 + t*8 + g] = idx[16g+r, it, t]
                jA = dram.tile([16, 1280], I16, tag="jA" + t)
                jAv = jA[:].rearrange("r (it t g) -> r it t g", it=8, t=KNN, g=8)
                for it in range(8):
                    idxf = pool.tile([128, KNN], F32, tag="idxf" + t)
                    nc.vector.tensor_copy(idxf[:], idx[:, it, 0:KNN])
                    it_ps = psum.tile([KNN, 128], F32, tag="aux")
                    nc.tensor.transpose(it_ps[:], idxf[:], ident[:])
                    idxw = pool.tile([KNN, 128], I16, tag="idxw" + t)
                    wv = idxw[:].rearrange("t (r g) -> t r g", r=16, g=8)
                    sv = it_ps[:].rearrange("t (g r) -> t r g", g=8, r=16)
                    nc.vector.tensor_copy(wv, sv)
                    nc.sync.dma_start(
                        jAv[:, it, :, :].rearrange("r t g -> t r g"), idxw[:])
                jsb = pool.tile([128, 1280], I16, tag="jsb" + t, bufs=1)
                for gg in range(8):
                    nc.sync.dma_start(jsb[16 * gg:16 * (gg + 1), :], jA[:])
                # gather (fp16) + 20-way max reduce per i-tile
                z = pool.tile([128, 8, O], F32, tag="z" + t, bufs=1)
                for it in range(8):
                    jslice = jsb[:, it * 160:(it + 1) * 160]
                    if O <= 128:
                        g_t = pool.tile([128, KNN, O], adt, tag="gath" + t)
                        nc.gpsimd.dma_gather(
                            out_ap=g_t[:], in_ap=at_dr[:], idxs_ap=jslice,
                            num_idxs=KNN * 128, num_idxs_reg=KNN * 128,
                            elem_size=O, single_packet=False)
                        nc.vector.tensor_reduce(
                            z[:, it, :], g_t[:].rearrange("p t o -> p o t"),
                            axis=mybir.AxisListType.X, op=ALU.max)
                    else:
                        for h in range(2):
                            g_t = pool.tile([128, KNN, 128], adt, tag="gath" + t)
                            nc.gpsimd.dma_gather(
                                out_ap=g_t[:],
                                in_ap=at_dr[:, h * 128:(h + 1) * 128],
                                idxs_ap=jslice,
                                num_idxs=KNN * 128, num_idxs_reg=KNN * 128,
                                elem_size=128, elem_step=O, single_packet=False)
                            nc.vector.tensor_reduce(
                                z[:, it, h * 128:(h + 1) * 128],
                                g_t[:].rearrange("p t o -> p o t"),
                                axis=mybir.AxisListType.X, op=ALU.max)
                # out = lrelu(z^T + (Wb@cur) + br): PE accumulates bT + z^T in
                # PSUM, Act evicts with Lrelu and per-channel bias.
                for it in range(8):
                    isl = slice(it * 128, (it + 1) * 128)
                    for h in range((O + 127) // 128):
                        oc = min(128, O - h * 128)
                        t_ps = psum.tile([128, 128], F32, tag="aux")
                        nc.tensor.matmul(t_ps[:oc, :],
                                         wbt[:, h * 128:h * 128 + oc],
                                         cur16[:, isl], start=True, stop=False)
                        nc.tensor.matmul(
                            t_ps[:oc, :], z[:, it, h * 128:h * 128 + oc],
                            ident[:], is_transpose=True, start=False, stop=True)
                        for dt_, roff in dests[h]:
                            nc.scalar.activation(
                                dt_[roff:roff + oc, it * 128:(it + 1) * 128],
                                t_ps[:oc, :], ACTF.Prelu,
                                bias=brt[0:oc, h:h + 1], alpha=0.2)

            for l, (C, O) in enumerate(LAYERS, 1):
                for s in range(bpc):
                    if l == 1:
                        x_sb = pool.tile([3, N], F32R, tag=f"x_in{s}", bufs=1)
                        nc.sync.dma_start(x_sb[:], x_in[s])
                        edge_layer(s, 1, x_sb[:], 3, 64, [[(xc[s][0], 0)]])
                    elif l == 2:
                        edge_layer(s, 2, xc[s][0][0:64, :], 64, 64,
                                   [[(xc[s][0], 64), (x2own[s], 0)]])
                    elif l == 3:
                        edge_layer(s, 3, x2own[s][:], 64, 128, [[(xc[s][1], 0)]])
                    else:
                        edge_layer(s, 4, xc[s][1][:], 128, 256,
                                   [[(xc[s][2], 0)], [(xc[s][3], 0)]])

            for s in range(bpc):
                # conv5 (f32r) + fused global max/mean pool
                accs = pool.tile([128, 8, 2], F32, tag=f"accs{s}")
                pmax = pool.tile([128, 8, 2], F32, tag=f"pmax{s}")
                for m in range(8):
                    for c in range(2):
                        y_ps = psum.tile([128, 512], F32, tag="dist")
                        for kt in range(4):
                            nc.tensor.matmul(
                                y_ps[:],
                                wsb["w5t"][:, kt, m * 128:(m + 1) * 128],
                                xc[s][kt][:, c * 512:(c + 1) * 512],
                                start=(kt == 0), stop=(kt == 3))
                        z5 = pool.tile([128, 512], F32, tag=f"z5{s}")
                        nc.scalar.activation(z5[:], y_ps[:], ACTF.Prelu,
                                             bias=wsb["b5c"][:, m:m + 1],
                                             alpha=0.2,
                                             accum_out=accs[:, m, c:c + 1])
                        nc.vector.tensor_reduce(
                            pmax[:, m, c:c + 1], z5[:],
                            axis=mybir.AxisListType.X, op=ALU.max)
                nc.vector.tensor_tensor(p2[:, 0:8, s], pmax[:, :, 0],
                                        pmax[:, :, 1], op=ALU.max)
                asm = pool.tile([128, 8], F32, tag=f"asm{s}")
                nc.vector.tensor_tensor(asm[:], accs[:, :, 0], accs[:, :, 1],
                                        op=ALU.add)
                nc.vector.tensor_scalar(
                    out=p2[:, 8:16, s], in0=asm[:], scalar1=1.0 / N,
                    scalar2=None, op0=ALU.mult)

            # ---- FC head (both samples batched) ----
            h1_ps = psum.tile([bpc, 512], F32, tag="fc")
            for t in range(16):
                w6kt = pool.tile([128, 512], F32R, tag="w6kt")
                nc.sync.dma_start(w6kt[:], win["w6t"][t])
                nc.tensor.matmul(h1_ps[:], p2[:, t, :],
                                 w6kt[:], start=(t == 0), stop=False)
            nc.tensor.matmul(h1_ps[:], ones[:, 0:bpc].bitcast(F32R),
                             wsb["b6r"][:].bitcast(F32R), start=False, stop=True)
            h1 = pool.tile([bpc, 512], F32, tag="h1")
            nc.scalar.copy(h1[:], h1_ps[:])
            nc.vector.scalar_tensor_tensor(out=h1[:], in0=h1[:], scalar=0.2,
                                           in1=h1[:], op0=ALU.mult, op1=ALU.max)
            h1t = pool.tile([128, 4, bpc], F32R, tag="h1t")
            for kt in range(4):
                t_ps = psum.tile([128, bpc], F32, tag="aux")
                nc.tensor.transpose(t_ps[:], h1[:, kt * 128:(kt + 1) * 128],
                                    ident[0:bpc, 0:bpc])
                nc.scalar.copy(h1t[:, kt, :], t_ps[:])
            h2_ps = psum.tile([bpc, 256], F32, tag="fc")
            for t in range(4):
                nc.tensor.matmul(h2_ps[:], h1t[:, t, :].bitcast(F32R),
                                 wsb["w7t"][:, t, :],
                                 start=(t == 0), stop=False)
            nc.tensor.matmul(h2_ps[:], ones[:, 0:bpc],
                             wsb["b7r"][:], start=False, stop=True)
            h2 = pool.tile([bpc, 256], F32, tag="h2")
            nc.scalar.copy(h2[:], h2_ps[:])
            nc.vector.scalar_tensor_tensor(out=h2[:], in0=h2[:], scalar=0.2,
                                           in1=h2[:], op0=ALU.mult, op1=ALU.max)
            h2t = pool.tile([128, 2, bpc], F32, tag="h2t")
            for kt in range(2):
                t_ps = psum.tile([128, bpc], F32, tag="aux")
                nc.tensor.transpose(t_ps[:], h2[:, kt * 128:(kt + 1) * 128],
                                    ident[0:bpc, 0:bpc])
                nc.scalar.copy(h2t[:, kt, :], t_ps[:])
            o_ps = psum.tile([bpc, 101], F32, tag="fc")
            for t in range(2):
                nc.tensor.matmul(o_ps[:], h2t[:, t, :], wsb["w8t"][:, t, 0:101],
                                 start=(t == 0), stop=False)
            nc.tensor.matmul(o_ps[:], ones[:, 0:bpc].bitcast(F32), wsb["b8r"][:],
                             start=False, stop=True)
            o_sb = pool.tile([bpc, 101], F32, tag="osb")
            nc.scalar.copy(o_sb[:], o_ps[:])
            nc.sync.dma_start(out[:], o_sb[:])

    nc.finalize()
    return nc


def prep_weights(inp):
    """Host-side: fold BN into weights; device-friendly layouts."""
    d = {}
    f32 = np.float32
    f16 = np.float16
    for l, (C, O) in enumerate(LAYERS, 1):
        w = np.asarray(inp[f"w{l}"], f32)
        g, b = np.asarray(inp[f"g{l}"], f32), np.asarray(inp[f"b{l}"], f32)
        m, v = np.asarray(inp[f"m{l}"], f32), np.asarray(inp[f"v{l}"], f32)
        s = g / np.sqrt(v + EPS)
        assert (s > 0).all(), "BN scale must be positive for the max/act swap"
        wn = w[:, :C] * s[:, None]
        wb = (w[:, C:] - w[:, :C]) * s[:, None]
        d[f"wnt{l}"] = np.ascontiguousarray(wn.T, f16)
        d[f"wbt{l}"] = np.ascontiguousarray(wb.T, f16)
        d[f"brt{l}"] = np.ascontiguousarray((b - m * s)[:, None], f32)
    s5 = np.asarray(inp["g5"], f32) / np.sqrt(np.asarray(inp["v5"], f32) + EPS)
    assert (s5 > 0).all()
    d["w5t"] = np.ascontiguousarray(
        (np.asarray(inp["w5"], f32) * s5[:, None]).T.reshape(4, 128, N), f32)
    d["b5c"] = np.ascontiguousarray(
        (np.asarray(inp["b5"], f32) - np.asarray(inp["m5"], f32) * s5)
        .reshape(8, 128).T, f32)
    s6 = np.asarray(inp["g6"], f32) / np.sqrt(np.asarray(inp["v6"], f32) + EPS)
    d["w6t"] = np.ascontiguousarray(
        (np.asarray(inp["wl1"], f32) * s6[:, None]).T.reshape(16, 128, 512), f32)
    d["b6r"] = np.ascontiguousarray(
        (np.asarray(inp["b6"], f32) - np.asarray(inp["m6"], f32) * s6)[None, :], f32)
    s7 = np.asarray(inp["g7"], f32) / np.sqrt(np.asarray(inp["v7"], f32) + EPS)
    d["w7t"] = np.ascontiguousarray(
        (np.asarray(inp["wl2"], f32) * s7[:, None]).T.reshape(4, 128, 256), f32)
    d["b7r"] = np.ascontiguousarray(
        (s7 * (np.asarray(inp["bl2"], f32) - np.asarray(inp["m7"], f32))
         + np.asarray(inp["b7"], f32))[None, :], f32)
    d["w8t"] = np.ascontiguousarray(
        np.asarray(inp["wl3"], f32).T.reshape(2, 128, 101), f32)
    d["b8r"] = np.ascontiguousarray(np.asarray(inp["bl3"], f32)[None, :], f32)
    d["ident"] = np.eye(128, dtype=f32)
    d["ident16"] = np.eye(128, dtype=f16)
    d["ones"] = np.ones((1, 128), f32)
    d["mhalf"] = np.full((128, 1), -0.5, f32)
    d["iota"] = np.broadcast_to(np.arange(N, dtype=np.int32)[None, :],
                                (128, N)).copy()
    d["qb"] = np.full((128, 1), 2.0 ** 19, f32)
    return d


_CACHE = {}


def _get_nc():
    if "nc" not in _CACHE:
        _CACHE["nc"] = build_nc()
    return _CACHE["nc"]


def kernel(**inputs):
    x = np.ascontiguousarray(np.asarray(inputs["x"], np.float32))
    assert x.shape == (16, 3, N), x.shape
    prep = prep_weights(inputs)
    nc = _get_nc()
    in_maps = []
    for c in range(NCORES):
        m = dict(prep)
        m["x"] = np.ascontiguousarray(x[c * BPC:(c + 1) * BPC])
        in_maps.append(m)
    res = bass_utils.run_bass_kernel_spmd(nc, in_maps, core_ids=list(range(NCORES)))
    out = np.concatenate([r["out"] for r in res.results], axis=0)
    return out.astype(np.float32)
